# revision 1
# baseline (speedup 1.0000x reference)
"""Trainium2 Bass kernel for the Koopman operator nn.Module.

Per-channel tiny MLPs (4 real channels, 6 complex-conjugate pairs, H=64,
2 hidden layers) over 65536 flattened batch elements, then a block-diagonal
Koopman update.  Pure data parallel over 8 NeuronCores (8192 elements each).

Per-core strategy:
  - elements on the free dim, hidden units on partitions
  - channels processed in block-diagonal PAIRS: every hidden matmul is a
    [128,128]x[128,512] f32r matmul at full PE stream rate
  - the 5 final-layer matmuls accumulate into ONE [128,512] PSUM tile with
    lambda at rows 0-3, mu at rows 32-37, omega at rows 64-69 so downstream
    reads start at legal partition bases (0/32/64)
  - layout churn (elem-major <-> channel-major) via PE transposes
  - exp/cos/sin on the scalar engine (cos via sin(x+pi/2)), combines on DVE
"""

import numpy as np

NR, NCC, L, H = 4, 6, 2, 64
B, S, C = 32, 2048, 16
NCORES = 8
F_CORE = B * S // NCORES        # 8192 elements per core
TILE = 512                      # elements per compute tile
GROUPS = TILE // 128            # 4
NT = F_CORE // TILE             # 16

HALF_PI = float(np.pi / 2)

_cached_nc = None


def _build():
    import concourse.tile as tile
    from concourse import bacc, mybir
    from concourse.masks import make_identity

    f32 = mybir.dt.float32
    f32r = mybir.dt.float32r
    RELU = mybir.ActivationFunctionType.Relu
    ADD = mybir.AluOpType.add
    MAX = mybir.AluOpType.max
    EXP = mybir.ActivationFunctionType.Exp
    SIN = mybir.ActivationFunctionType.Sin

    nc = bacc.Bacc("TRN2", target_bir_lowering=False, debug=False,
                   num_devices=NCORES)

    z = nc.dram_tensor("z", [F_CORE, C], f32, kind="ExternalInput").ap()
    w0p = nc.dram_tensor("w0p", [5, 10, 128], f32r, kind="ExternalInput").ap()
    b0p = nc.dram_tensor("b0p", [5, 128, 1], f32, kind="ExternalInput").ap()
    wmp = nc.dram_tensor("wmp", [L, 5, 128, 128], f32r, kind="ExternalInput").ap()
    bmp = nc.dram_tensor("bmp", [L, 5, 128, 1], f32, kind="ExternalInput").ap()
    wlp = nc.dram_tensor("wlp", [5, 128, 128], f32r, kind="ExternalInput").ap()
    blr4 = nc.dram_tensor("blr4", [4, 1], f32, kind="ExternalInput").ap()
    blmu = nc.dram_tensor("blmu", [6, 1], f32, kind="ExternalInput").ap()
    blom = nc.dram_tensor("blom", [6, 1], f32, kind="ExternalInput").ap()
    blompi = nc.dram_tensor("blompi", [6, 1], f32, kind="ExternalInput").ap()
    out = nc.dram_tensor("out", [F_CORE, C], f32, kind="ExternalOutput").ap()

    z_r = z.rearrange("(t g p) c -> t p g c", g=GROUPS, p=128)
    out_r = out.rearrange("(t g p) c -> t p g c", g=GROUPS, p=128)

    with tile.TileContext(nc) as tc:
        with (
            tc.tile_pool(name="singles", bufs=1) as singles,
            tc.tile_pool(name="io", bufs=4) as io,
            tc.tile_pool(name="acts", bufs=8) as acts,
            tc.tile_pool(name="pshid", bufs=3, space="PSUM") as pshid,
            tc.tile_pool(name="psxT", bufs=2, space="PSUM") as psxT,
            tc.tile_pool(name="pstk", bufs=2, space="PSUM") as pstk,
            tc.tile_pool(name="psT", bufs=1, space="PSUM") as pstp,
        ):
            ident = singles.tile([128, 128], f32, tag="ident")
            make_identity(nc, ident)

            # --- load weights (replicated; packed block-diagonal on host) ---
            w0_sb, b0_sb, wm_sb, bm_sb, wl_sb = [], [], [], [], []
            for j in range(5):
                w = singles.tile([10, 128], f32r, tag=f"w0_{j}")
                nc.sync.dma_start(out=w, in_=w0p[j])
                w0_sb.append(w)
                b = singles.tile([128, 1], f32, tag=f"b0_{j}")
                nc.sync.dma_start(out=b, in_=b0p[j])
                b0_sb.append(b)
            for l in range(L):
                wm_sb.append([])
                bm_sb.append([])
                for j in range(5):
                    w = singles.tile([128, 128], f32r, tag=f"wm_{l}_{j}")
                    nc.sync.dma_start(out=w, in_=wmp[l, j])
                    wm_sb[l].append(w)
                    b = singles.tile([128, 1], f32, tag=f"bm_{l}_{j}")
                    nc.sync.dma_start(out=b, in_=bmp[l, j])
                    bm_sb[l].append(b)
            for j in range(5):
                w = singles.tile([128, 128], f32r, tag=f"wl_{j}")
                nc.sync.dma_start(out=w, in_=wlp[j])
                wl_sb.append(w)
            blr4_sb = singles.tile([4, 1], f32, tag="blr4")
            nc.sync.dma_start(out=blr4_sb, in_=blr4)
            blmu_sb = singles.tile([6, 1], f32, tag="blmu")
            nc.sync.dma_start(out=blmu_sb, in_=blmu)
            blom_sb = singles.tile([6, 1], f32, tag="blom")
            nc.sync.dma_start(out=blom_sb, in_=blom)
            blompi_sb = singles.tile([6, 1], f32, tag="blompi")
            nc.sync.dma_start(out=blompi_sb, in_=blompi)

            # --- main loop over 512-element tiles ---
            for t in range(NT):
                z_nat = io.tile([128, GROUPS, C], f32, tag="z_nat")
                nc.sync.dma_start(out=z_nat, in_=z_r[t])
                z1 = z_nat[:, :, 4:16:2]
                z2 = z_nat[:, :, 5:16:2]

                # x_nat: [zr(4) | z_mag(6)] per group, elem-major
                x_nat = io.tile([128, GROUPS, 10], f32, tag="x_nat")
                nc.vector.tensor_copy(x_nat[:, :, 0:4], z_nat[:, :, 0:4])
                zm1 = io.tile([128, GROUPS, 6], f32, tag="zm1")
                zm2 = io.tile([128, GROUPS, 6], f32, tag="zm2")
                nc.vector.tensor_mul(zm1, z1, z1)
                nc.vector.tensor_mul(zm2, z2, z2)
                nc.vector.tensor_add(x_nat[:, :, 4:10], zm1, zm2)

                # transpose to channel-major xT [10, 512] (f32r for matmul rhs)
                xT_ps = psxT.tile([10, TILE], f32, tag="xT")
                for g in range(GROUPS):
                    nc.tensor.transpose(
                        xT_ps[:, g * 128:(g + 1) * 128], x_nat[:, g, :], ident)
                xT = acts.tile([10, TILE], f32r, tag="xT_sb")
                nc.vector.tensor_copy(xT, xT_ps)

                # per-pair MLPs; final matmuls accumulate into stk_ps:
                # rows 0-3 lambda, rows 32-37 mu, rows 64-69 omega
                stk_ps = pstk.tile([128, TILE], f32, tag="stk")
                for j in range(5):
                    ps0 = pshid.tile([128, TILE], f32, tag="ps")
                    nc.tensor.matmul(ps0, w0_sb[j], xT, start=True, stop=True)
                    h = acts.tile([128, TILE], f32r, tag="h")
                    nc.vector.tensor_scalar(h, ps0, b0_sb[j], 0.0, ADD, MAX)
                    for l in range(L):
                        psl = pshid.tile([128, TILE], f32, tag="ps")
                        nc.tensor.matmul(psl, wm_sb[l][j], h,
                                         start=True, stop=True)
                        h = acts.tile([128, TILE], f32r, tag="h")
                        if j >= 3 and l == 0:
                            nc.vector.tensor_scalar(h, psl, bm_sb[l][j],
                                                    0.0, ADD, MAX)
                        else:
                            nc.scalar.activation(h, psl, RELU, bias=bm_sb[l][j])
                    nc.tensor.matmul(stk_ps, wl_sb[j], h,
                                     start=(j == 0), stop=(j == 4))

                # lambda += bl_r; mc = exp(mu)*cos(om); ms = exp(mu)*sin(om)
                lam = acts.tile([4, TILE], f32, tag="lam")
                nc.vector.tensor_scalar_add(lam, stk_ps[0:4], blr4_sb)
                e = acts.tile([6, TILE], f32, tag="e")
                nc.scalar.activation(e, stk_ps[32:38], EXP, bias=blmu_sb)
                cs = acts.tile([6, TILE], f32, tag="cs")
                nc.scalar.activation(cs, stk_ps[64:70], SIN, bias=blompi_sb)
                sn = acts.tile([6, TILE], f32, tag="sn")
                nc.scalar.activation(sn, stk_ps[64:70], SIN, bias=blom_sb)
                mc = acts.tile([6, TILE], f32, tag="mc")
                ms = acts.tile([6, TILE], f32, tag="ms")
                nc.vector.tensor_mul(mc, e, cs)
                nc.vector.tensor_mul(ms, e, sn)

                # transpose back to elem-major T_ps [128, g, 16]:
                # cols 0-3 lambda, 4-9 mc, 10-15 ms
                T_ps = pstp.tile([128, GROUPS, 16], f32, tag="T")
                for g in range(GROUPS):
                    gs = slice(g * 128, (g + 1) * 128)
                    nc.tensor.transpose(T_ps[:, g, 0:4], lam[:, gs],
                                        ident[0:4, 0:4])
                    nc.tensor.transpose(T_ps[:, g, 4:10], mc[:, gs],
                                        ident[0:6, 0:6])
                    nc.tensor.transpose(T_ps[:, g, 10:16], ms[:, gs],
                                        ident[0:6, 0:6])

                # combine: out_r = zr*lam; o1 = z1*mc + z2*ms; o2 = z2*mc - z1*ms
                lamT = T_ps[:, :, 0:4]
                mcT = T_ps[:, :, 4:10]
                msT = T_ps[:, :, 10:16]
                o_nat = io.tile([128, GROUPS, C], f32, tag="o_nat")
                t1 = io.tile([128, GROUPS, 6], f32, tag="t1")
                t2 = io.tile([128, GROUPS, 6], f32, tag="t2")
                nc.vector.tensor_mul(o_nat[:, :, 0:4], z_nat[:, :, 0:4], lamT)
                nc.vector.tensor_mul(t1, z1, mcT)
                nc.vector.tensor_mul(t2, z2, msT)
                nc.vector.tensor_add(o_nat[:, :, 4:16:2], t1, t2)
                nc.vector.tensor_mul(t1, z2, mcT)
                nc.vector.tensor_mul(t2, z1, msT)
                nc.vector.tensor_sub(o_nat[:, :, 5:16:2], t1, t2)

                nc.sync.dma_start(out=out_r[t], in_=o_nat)

    nc.compile()
    return nc


def _pack_weights(i):
    """Pack per-channel weights into block-diagonal pair form."""
    f32 = np.float32
    W0_r, b0_r = np.asarray(i["W0_r"], f32), np.asarray(i["b0_r"], f32)
    Wm_r, bm_r = np.asarray(i["Wm_r"], f32), np.asarray(i["bm_r"], f32)
    Wl_r, bl_r = np.asarray(i["Wl_r"], f32), np.asarray(i["bl_r"], f32)
    W0_c, b0_c = np.asarray(i["W0_c"], f32), np.asarray(i["b0_c"], f32)
    Wm_c, bm_c = np.asarray(i["Wm_c"], f32), np.asarray(i["bm_c"], f32)
    Wl_c, bl_c = np.asarray(i["Wl_c"], f32), np.asarray(i["bl_c"], f32)

    w0p = np.zeros((5, 10, 128), f32)
    b0p = np.zeros((5, 128), f32)
    wmp = np.zeros((L, 5, 128, 128), f32)
    bmp = np.zeros((L, 5, 128), f32)
    wlp = np.zeros((5, 128, 128), f32)
    for j in range(5):
        if j < 2:
            a, b = 2 * j, 2 * j + 1
            W0, b0, Wm, bm = W0_r, b0_r, Wm_r, bm_r
        else:
            a, b = 2 * (j - 2), 2 * (j - 2) + 1
            W0, b0, Wm, bm = W0_c, b0_c, Wm_c, bm_c
        r0 = 2 * j if j < 2 else 4 + 2 * (j - 2)
        w0p[j, r0, 0:64] = W0[a]
        w0p[j, r0 + 1, 64:128] = W0[b]
        b0p[j, 0:64] = b0[a]
        b0p[j, 64:128] = b0[b]
        for l in range(L):
            wmp[l, j, 0:64, 0:64] = Wm[l, a]
            wmp[l, j, 64:128, 64:128] = Wm[l, b]
            bmp[l, j, 0:64] = bm[l, a]
            bmp[l, j, 64:128] = bm[l, b]
        if j < 2:
            wlp[j, 0:64, 2 * j] = Wl_r[a][:, 0]
            wlp[j, 64:128, 2 * j + 1] = Wl_r[b][:, 0]
        else:
            jc = j - 2
            wlp[j, 0:64, 32 + 2 * jc] = Wl_c[a][:, 0]       # mu_a
            wlp[j, 64:128, 33 + 2 * jc] = Wl_c[b][:, 0]     # mu_b
            wlp[j, 0:64, 64 + 2 * jc] = Wl_c[a][:, 1]       # om_a
            wlp[j, 64:128, 65 + 2 * jc] = Wl_c[b][:, 1]     # om_b

    blom = bl_c[:, 1:2].copy()
    return {"w0p": w0p, "b0p": b0p[:, :, None], "wmp": wmp,
            "bmp": bmp[:, :, :, None], "wlp": wlp,
            "blr4": bl_r[:, 0:1].copy(), "blmu": bl_c[:, 0:1].copy(),
            "blom": blom, "blompi": blom + np.float32(HALF_PI)}


def kernel(**inputs):
    global _cached_nc
    if _cached_nc is None:
        _cached_nc = _build()
    nc = _cached_nc

    from concourse.bass_utils import run_bass_kernel_spmd

    weights = _pack_weights(inputs)
    z = np.ascontiguousarray(np.asarray(inputs["z"], np.float32)
                             .reshape(NCORES, F_CORE, C))
    in_maps = [dict(weights, z=z[i]) for i in range(NCORES)]
    res = run_bass_kernel_spmd(nc, in_maps, core_ids=list(range(NCORES)))
    outs = [np.asarray(res.results[i]["out"]) for i in range(NCORES)]
    return np.concatenate(outs, axis=0).reshape(B, S, C)



# revision 9
# speedup vs baseline: 2.4000x; 2.4000x over previous
"""Trainium2 Bass kernel for the Koopman operator nn.Module.

Per-channel tiny MLPs (4 real channels, 6 complex-conjugate pairs, H=64,
2 hidden layers) over 65536 flattened batch elements, then a block-diagonal
Koopman update.  Pure data parallel over 8 NeuronCores (8192 elements each).

v2 strategy (weight-stationary, fp16 matmul path, transpose-free input):
  - host uploads z in three layouts: elem-major [128, 64, 16] for the final
    combine, strip-packed channel-major z1/z2 (at partition bases 0/32/64/96)
    for the on-device |z|^2, and fp16 zr rows DMA'd straight into the MLP
    input tile -- no input transposes on the tensor engine
  - all matmuls fp16 (1 cycle/row) with fp32 PSUM accumulation
  - 4 quarters x (L0 / hid0 / hid1 / final phases across all 5 pair-blocks):
    consecutive matmuls share stationary weights and the program order lets
    pair j's ReLUs drain while pairs j+1.. stream, keeping the PE p-state
    ramped at 2.4 GHz
  - ReLUs round-robin over DVE / Act / GpSimd (three engines)
  - final-layer outputs go back to elem-major via 4 PE transposes per chunk;
    exp/sin/combine run as a handful of big batched ops at the end
    (sin(x+pi/2) for cos); activation table loads ~2 for the whole kernel
"""

import numpy as np

NR, NCC, L, H = 4, 6, 2, 64
B, S, C = 32, 2048, 16
NCORES = 8
F_CORE = B * S // NCORES        # 8192 elements per core
CHUNK = 512                     # elements per matmul chunk (one PSUM bank)
NCH = F_CORE // CHUNK           # 16 chunks
NQ = 4                          # quarters
KPQ = NCH // NQ                 # 4 chunks per quarter
NSTRIP = 2                      # xcat partition bands at bases 0 and 64
STRIP = F_CORE // NSTRIP        # 4096 elements per strip
CPS = STRIP // CHUNK            # 8 chunks per strip

HALF_PI = float(np.pi / 2)

# wcat column layout (fp16): w0(5x128) | wm0(5x128) | wm1(5x128) | wl(5x128)
# | ident(128)
W0_OFF, WM0_OFF, WM1_OFF, WL_OFF, ID_OFF = 0, 640, 1280, 1920, 2560
WCAT_COLS = 2688
# bcat column layout (fp32): b0(5) | bm0(5) | bm1(5) | bias128(1) | pi/2(1)
BCAT_COLS = 17

_cached_nc = None


def _build():
    import concourse.tile as tile
    from concourse import bacc, mybir

    f32 = mybir.dt.float32
    f16 = mybir.dt.float16
    RELU = mybir.ActivationFunctionType.Relu
    IDENT = mybir.ActivationFunctionType.Identity
    SQUARE = mybir.ActivationFunctionType.Square
    EXP = mybir.ActivationFunctionType.Exp
    SIN = mybir.ActivationFunctionType.Sin
    ADD = mybir.AluOpType.add
    MAX = mybir.AluOpType.max

    nc = bacc.Bacc("TRN2", target_bir_lowering=False, debug=False,
                   num_devices=NCORES)

    zin_d = nc.dram_tensor("zin", [128, NCH, KPQ, C], f32,
                           kind="ExternalInput").ap()
    z12_d = nc.dram_tensor("z12", [2, 128, STRIP], f32,
                           kind="ExternalInput").ap()
    xzr_d = nc.dram_tensor("xzr", [NSTRIP, 4, STRIP], f16,
                           kind="ExternalInput").ap()
    wcat_d = nc.dram_tensor("wcat", [128, WCAT_COLS], f16,
                            kind="ExternalInput").ap()
    bcat_d = nc.dram_tensor("bcat", [128, BCAT_COLS], f32,
                            kind="ExternalInput").ap()
    out_d = nc.dram_tensor("out", [128, NCH, KPQ, C], f32,
                           kind="ExternalOutput").ap()

    with tile.TileContext(nc) as tc:
        with (
            tc.tile_pool(name="singles", bufs=1) as singles,
            tc.tile_pool(name="scratch", bufs=1) as scratch,
            tc.tile_pool(name="hps", bufs=1) as hps,
            tc.tile_pool(name="pshid", bufs=3, space="PSUM") as pshid,
            tc.tile_pool(name="pstk", bufs=1, space="PSUM") as pstk,
            tc.tile_pool(name="pstp", bufs=1, space="PSUM") as pstp,
        ):
            # ---- uploads ----
            wcat = singles.tile([128, WCAT_COLS], f16, tag="wcat")
            nc.sync.dma_start(out=wcat, in_=wcat_d)
            bcat = singles.tile([128, BCAT_COLS], f32, tag="bcat")
            nc.sync.dma_start(out=bcat, in_=bcat_d)
            zin = singles.tile([128, NCH, KPQ, C], f32, tag="zin")
            nc.sync.dma_start(out=zin, in_=zin_d)
            z1c = singles.tile([128, STRIP], f32, tag="z1c")
            nc.sync.dma_start(out=z1c, in_=z12_d[0])
            z2c = singles.tile([128, STRIP], f32, tag="z2c")
            nc.sync.dma_start(out=z2c, in_=z12_d[1])

            # ---- x = [zmag(6) ; zr(4)] per 32-partition strip band ----
            xcat = singles.tile([128, STRIP], f16, tag="xcat")
            sq1 = scratch.tile([128, STRIP], f32, tag="sq1")
            sq2 = scratch.tile([128, STRIP], f32, tag="sq2")
            nc.scalar.activation(sq1, z1c, SQUARE)
            nc.scalar.activation(sq2, z2c, SQUARE)
            nc.vector.tensor_add(xcat, sq1, sq2)
            for s in range(NSTRIP):
                nc.sync.dma_start(out=xcat[64 * s + 6:64 * s + 10],
                                  in_=xzr_d[s])

            ident = wcat[:, ID_OFF:ID_OFF + 128]
            bias128 = bcat[:, 15:16]

            # full-width staging for the post phase
            t_all = singles.tile([128, NCH, CHUNK], f16, tag="t_all")
            o_full = singles.tile([128, NCH, KPQ, C], f32, tag="o_full")

            # ReLU engine round-robin: weighted DVE/Act/Pool
            relu_seq = []

            def relu(h, ps, bias_ap):
                i = len(relu_seq) % 2
                relu_seq.append(0)
                if i == 0:
                    nc.vector.tensor_scalar(h, ps, bias_ap, 0.0, ADD, MAX)
                else:
                    nc.scalar.activation(h, ps, RELU, bias=bias_ap)

            # ---- MLP: 4 quarters, weight-stationary phases ----
            for q in range(NQ):
                s = (q * KPQ) // CPS
                rs = slice(64 * s, 64 * s + 10)
                h0, h1, h2 = {}, {}, {}
                # layer 0
                for j in range(5):
                    w = wcat[rs, W0_OFF + j * 128:W0_OFF + (j + 1) * 128]
                    for kk in range(KPQ):
                        k = q * KPQ + kk
                        cc = (k % CPS) * CHUNK
                        ps = pshid.tile([128, CHUNK], f32, tag="ps")
                        nc.tensor.matmul(
                            ps, w, xcat[rs, cc:cc + CHUNK],
                            start=True, stop=True)
                        h = hps.tile([128, CHUNK], f16, tag=f"h0_{j}_{kk}")
                        relu(h, ps, bcat[:, j:j + 1])
                        h0[j, kk] = h
                # hidden layers
                for l, (off, hin, hout) in enumerate(
                        ((WM0_OFF, h0, h1), (WM1_OFF, h1, h2))):
                    for j in range(5):
                        w = wcat[:, off + j * 128:off + (j + 1) * 128]
                        b = bcat[:, 5 + 5 * l + j:6 + 5 * l + j]
                        for kk in range(KPQ):
                            ps = pshid.tile([128, CHUNK], f32, tag="ps")
                            nc.tensor.matmul(ps, w, hin[j, kk],
                                             start=True, stop=True)
                            h = hps.tile([128, CHUNK], f16,
                                         tag=f"h{l + 1}_{j}_{kk}")
                            relu(h, ps, b)
                            hout[j, kk] = h
                # final layer: disjoint output rows per j into one bank/chunk
                stks = []
                for j in range(5):
                    w = wcat[:, WL_OFF + j * 128:WL_OFF + (j + 1) * 128]
                    for kk in range(KPQ):
                        if j == 0:
                            stk_t = pstk.tile([128, CHUNK], f32,
                                              tag=f"stk_{kk}")
                            stks.append(stk_t)
                        nc.tensor.matmul(stks[kk], w, h2[j, kk],
                                         start=(j == 0), stop=(j == 4))
                # post per chunk: +bias, fp16, transpose to elem-major
                for kk in range(KPQ):
                    k = q * KPQ + kk
                    sstk = hps.tile([128, CHUNK], f16, tag=f"sstk_{kk}")
                    nc.scalar.activation(sstk, stks[kk], IDENT, bias=bias128)
                    tp = pstp.tile([128, CHUNK], f16, tag="tp")
                    for g in range(KPQ):
                        nc.tensor.transpose(
                            tp[:, g * 128:(g + 1) * 128],
                            sstk[:, g * 128:(g + 1) * 128], ident)
                    nc.vector.tensor_copy(t_all[:, k], tp)

            # ---- post: big batched ops over all 16 chunks ----
            # t_all cols within group g: 0-3 lam | 32-37 mu | 64-69 om
            t4 = t_all.rearrange("p k (g c) -> p k g c", g=KPQ, c=128)
            lamT = t4[:, :, :, 0:4]
            muT = t4[:, :, :, 32:38]
            omT = t4[:, :, :, 64:70]

            e_f = singles.tile([128, NCH, KPQ, 6], f32, tag="e_f")
            cs_f = singles.tile([128, NCH, KPQ, 6], f32, tag="cs_f")
            sn_f = singles.tile([128, NCH, KPQ, 6], f32, tag="sn_f")
            nc.scalar.activation(e_f, muT, EXP)
            nc.scalar.activation(cs_f, omT, SIN, bias=bcat[:, 16:17])
            nc.scalar.activation(sn_f, omT, SIN)
            mc_f = singles.tile([128, NCH, KPQ, 6], f32, tag="mc_f")
            ms_f = singles.tile([128, NCH, KPQ, 6], f32, tag="ms_f")
            nc.vector.tensor_mul(mc_f, e_f, cs_f)
            nc.vector.tensor_mul(ms_f, e_f, sn_f)

            zr_v = zin[:, :, :, 0:4]
            z1_v = zin[:, :, :, 4:16:2]
            z2_v = zin[:, :, :, 5:16:2]
            t1f = scratch.tile([128, NCH, KPQ, 6], f32, tag="t1f")
            t2f = scratch.tile([128, NCH, KPQ, 6], f32, tag="t2f")
            nc.vector.tensor_mul(o_full[:, :, :, 0:4], zr_v, lamT)
            nc.vector.tensor_mul(t1f, z1_v, mc_f)
            nc.vector.tensor_mul(t2f, z2_v, ms_f)
            nc.vector.tensor_add(o_full[:, :, :, 4:16:2], t1f, t2f)
            nc.vector.tensor_mul(t1f, z2_v, mc_f)
            nc.vector.tensor_mul(t2f, z1_v, ms_f)
            nc.vector.tensor_sub(o_full[:, :, :, 5:16:2], t1f, t2f)

            nc.sync.dma_start(out=out_d, in_=o_full)

    nc.compile()
    return nc


def _pack_weights(i):
    """Pack per-channel weights into the fused fp16 wcat / fp32 bcat blocks."""
    f32, f16 = np.float32, np.float16
    W0_r, b0_r = np.asarray(i["W0_r"], f32), np.asarray(i["b0_r"], f32)
    Wm_r, bm_r = np.asarray(i["Wm_r"], f32), np.asarray(i["bm_r"], f32)
    Wl_r, bl_r = np.asarray(i["Wl_r"], f32), np.asarray(i["bl_r"], f32)
    W0_c, b0_c = np.asarray(i["W0_c"], f32), np.asarray(i["b0_c"], f32)
    Wm_c, bm_c = np.asarray(i["Wm_c"], f32), np.asarray(i["bm_c"], f32)
    Wl_c, bl_c = np.asarray(i["Wl_c"], f32), np.asarray(i["bl_c"], f32)

    wcat = np.zeros((128, WCAT_COLS), f16)
    bcat = np.zeros((128, BCAT_COLS), f32)
    for j in range(5):
        if j < 2:
            a, b = 2 * j, 2 * j + 1
            W0, b0, Wm, bm = W0_r, b0_r, Wm_r, bm_r
            xra, xrb = 6 + a, 6 + b          # zr rows of x
        else:
            a, b = 2 * (j - 2), 2 * (j - 2) + 1
            W0, b0, Wm, bm = W0_c, b0_c, Wm_c, bm_c
            xra, xrb = a, b                  # zmag rows of x
        # layer 0, replicated at each strip base (partitions 0 and 64)
        for s in range(NSTRIP):
            wcat[64 * s + xra, W0_OFF + j * 128:W0_OFF + j * 128 + 64] = W0[a]
            wcat[64 * s + xrb, W0_OFF + j * 128 + 64:W0_OFF + (j + 1) * 128] \
                = W0[b]
        bcat[0:64, j] = b0[a]
        bcat[64:128, j] = b0[b]
        # hidden layers, block diagonal
        for l, off in enumerate((WM0_OFF, WM1_OFF)):
            wcat[0:64, off + j * 128:off + j * 128 + 64] = Wm[l, a]
            wcat[64:128, off + j * 128 + 64:off + (j + 1) * 128] = Wm[l, b]
            bcat[0:64, 5 + 5 * l + j] = bm[l, a]
            bcat[64:128, 5 + 5 * l + j] = bm[l, b]
        # final layer -> rows 0-3 lam, 32-37 mu, 64-69 om
        wo = WL_OFF + j * 128
        if j < 2:
            wcat[0:64, wo + 2 * j] = Wl_r[a][:, 0]
            wcat[64:128, wo + 2 * j + 1] = Wl_r[b][:, 0]
        else:
            jc = j - 2
            wcat[0:64, wo + 32 + 2 * jc] = Wl_c[a][:, 0]
            wcat[64:128, wo + 33 + 2 * jc] = Wl_c[b][:, 0]
            wcat[0:64, wo + 64 + 2 * jc] = Wl_c[a][:, 1]
            wcat[64:128, wo + 65 + 2 * jc] = Wl_c[b][:, 1]
    wcat[:, ID_OFF:ID_OFF + 128] = np.eye(128, dtype=f16)
    bcat[:, 16] = HALF_PI
    bcat[0:4, 15] = bl_r[:, 0]
    bcat[32:38, 15] = bl_c[:, 0]
    bcat[64:70, 15] = bl_c[:, 1]
    return {"wcat": wcat, "bcat": bcat}


def _pack_z(z_core):
    """Per-core z [8192, 16] -> zin / z12 / xzr DRAM layouts."""
    f32, f16 = np.float32, np.float16
    zc = np.asarray(z_core, f32)
    zin = np.ascontiguousarray(
        zc.reshape(64, 128, C).transpose(1, 0, 2)).reshape(128, NCH, KPQ, C)
    z1 = zc[:, 4:16:2].reshape(NSTRIP, STRIP, 6)   # [s, e, ch]
    z2 = zc[:, 5:16:2].reshape(NSTRIP, STRIP, 6)
    z12 = np.zeros((2, 128, STRIP), f32)
    for s in range(NSTRIP):
        z12[0, 64 * s:64 * s + 6] = z1[s].T
        z12[1, 64 * s:64 * s + 6] = z2[s].T
    xzr = np.ascontiguousarray(
        zc[:, 0:4].reshape(NSTRIP, STRIP, 4).transpose(0, 2, 1)).astype(f16)
    return {"zin": zin, "z12": z12, "xzr": xzr}


def kernel(**inputs):
    global _cached_nc
    if _cached_nc is None:
        _cached_nc = _build()
    nc = _cached_nc

    from concourse.bass_utils import run_bass_kernel_spmd

    weights = _pack_weights(inputs)
    z = np.asarray(inputs["z"], np.float32).reshape(NCORES, F_CORE, C)
    in_maps = [dict(weights, **_pack_z(z[i])) for i in range(NCORES)]
    res = run_bass_kernel_spmd(nc, in_maps, core_ids=list(range(NCORES)))
    outs = [
        np.asarray(res.results[i]["out"])
        .reshape(128, 64, C).transpose(1, 0, 2).reshape(F_CORE, C)
        for i in range(NCORES)
    ]
    return np.concatenate(outs, axis=0).reshape(B, S, C)
